# revision 14
# baseline (speedup 1.0000x reference)
"""Trainium2 Bass kernel for the DEN (Mahalanobis distance) layer.

Computes out[b, e] = (x_b - c_e)^T Sigma_e^{-1} (x_b - c_e) for
x [8192, 128], Centroids [128, 1, 128], Sigmas [128, 128, 128].

Strategy
--------
Expand the quadratic form with A_e = sym(Sigma_e^{-1}) (the quadratic form
only sees the symmetric part):

    out[b, e] = x_b^T A_e x_b - 2 (A_e c_e) . x_b + c_e^T A_e c_e

and decompose x^T A x over the wrapped diagonals of A:

    x^T A_e x = sum_{j=0..64} sum_d s_j A_e[d, (d+j)%128] * x_d * x_{(d+j)%128}

(s_j = 2 for 1<=j<=63, else 1: each unordered pair lands in exactly one
wrapped diagonal j<=63, and diagonal 64 visits its pairs twice).

The shifted products P_j[d, b] = xT[d, b] * xT[(d+j)%128, b] feed a chain
of PSUM-accumulated [128,128]x[128,512] matmuls with host-precomputed
coefficient packs, one per diagonal, plus one matmul for the linear term;
the constant term rides in as the activation bias during PSUM->SBUF
eviction.

Compute-engine instructions need all operands on the same partitions
(lanes are hardwired per partition), so the shifts come from host-prebuilt
partition-rotated copies of xT: a difference set of rotations
{0..7, 8, 16, ..., 64} covers every j = b - a in 0..64, and the row
rotation by a_j is absorbed into the coefficient packs on the host.  All
rotations live in one SBUF mega-tile, so diagonals sharing an operand
merge into a single strided vector-engine op: the whole product stream is
9 tensor_mul ops + 1 scalar-engine square (j=0).

Sharding: data-parallel over batch B across the 8 cores (1024 rows each);
coefficient packs (derived from Sigmas/Centroids) are replicated.
"""

import os
import sys

sys.path.insert(0, "/opt/trn_rl_repo")

import numpy as np
import ml_dtypes

E, B, D = 128, 8192, 128
NCORES = 8
BLOC = B // NCORES          # 1024 batch rows per core
BT = 512                    # matmul free-dim tile (one PSUM bank)
NPACK = 65                  # wrapped diagonals j = 0..64
NSLOT = 20                  # rotation slots: 0..3 then 4,8,...,64
ROTVALS = (1, 2, 3) + tuple(range(4, 65, 4))
CHUNK = 5                   # coefficient packs per DMA chunk
NCHUNK = 13                 # 65 / 5

# pack emission order: j=0 (scalar-engine square), then 16 groups of 4:
# group k holds j = 4(k+1) - i for i = 0..3 (in0 = rotations 0..3,
# in1 = rotation 4(k+1) broadcast)
ORDER = [0] + [4 * (k + 1) - i for k in range(16) for i in range(4)]

bf16 = ml_dtypes.bfloat16

_STATE: dict = {}


def _patch_ldw_opt():
    """Let walrus hoist/dedupe LDWEIGHTS (off by default in this harness)."""
    from concourse import bass_utils

    if getattr(bass_utils, "_ldw_patched", False):
        return
    orig = bass_utils.get_walrus_args

    def patched(*a, **k):
        return [x.replace("--enable-ldw-opt=false", "--enable-ldw-opt=true")
                for x in orig(*a, **k)]

    bass_utils.get_walrus_args = patched
    bass_utils._ldw_patched = True


def _build_module():
    import concourse.bacc as bacc
    import concourse.tile as tile
    import concourse.mybir as mybir
    from contextlib import ExitStack

    nc = bacc.Bacc("TRN2", target_bir_lowering=False, debug=False)

    xT_d = nc.dram_tensor("xT", [D, BLOC], mybir.dt.bfloat16, kind="ExternalInput")
    xr_d = nc.dram_tensor("xrot", [NSLOT - 1, D, BLOC], mybir.dt.bfloat16,
                          kind="ExternalInput")
    cw_d = nc.dram_tensor("cw", [D, NPACK * E], mybir.dt.bfloat16, kind="ExternalInput")
    um_d = nc.dram_tensor("um", [D, E], mybir.dt.bfloat16, kind="ExternalInput")
    tv_d = nc.dram_tensor("tv", [E, 1], mybir.dt.float32, kind="ExternalInput")
    out_d = nc.dram_tensor("out", [E, BLOC], mybir.dt.float32, kind="ExternalOutput")

    f32 = mybir.dt.float32
    b16 = mybir.dt.bfloat16
    Ident = mybir.ActivationFunctionType.Identity

    with tile.TileContext(nc) as tc, ExitStack() as ctx:
        const_pool = ctx.enter_context(tc.tile_pool(name="const", bufs=1))
        coef_pool = ctx.enter_context(tc.tile_pool(name="coef", bufs=NCHUNK))
        p0_pool = ctx.enter_context(tc.tile_pool(name="p0", bufs=1))
        g_pool = ctx.enter_context(tc.tile_pool(name="g", bufs=4))
        psum_pool = ctx.enter_context(tc.tile_pool(name="acc", bufs=1, space="PSUM"))
        out_pool = ctx.enter_context(tc.tile_pool(name="outs", bufs=2))

        ROTS = const_pool.tile([D, NSLOT * BLOC], b16, tag="rots")
        R3 = ROTS[:, :].rearrange("p (s b) -> p s b", s=NSLOT)
        UM = const_pool.tile([D, E], b16, tag="um")
        TV = const_pool.tile([E, 1], f32, tag="tv")
        coef_tiles = [coef_pool.tile([D, CHUNK * E], b16, name=f"cw{ci}", tag="cw")
                      for ci in range(NCHUNK)]

        # DMA emission in consumption order, rotations interleaved with
        # coefficient chunks so neither engine starves at the head
        nc.sync.dma_start(ROTS[:, 0:BLOC], xT_d.ap())
        nc.sync.dma_start(UM[:], um_d.ap())
        nc.sync.dma_start(TV[:], tv_d.ap())

        def dma_rot(s):
            nc.sync.dma_start(ROTS[:, s * BLOC:(s + 1) * BLOC], xr_d.ap()[s - 1])

        def dma_coef(ci):
            nc.sync.dma_start(coef_tiles[ci][:],
                              cw_d.ap()[:, ci * CHUNK * E:(ci + 1) * CHUNK * E])

        half = BLOC // 2
        for s0 in (1, 2, 3, 4):
            nc.sync.dma_start(ROTS[:, s0 * BLOC:s0 * BLOC + half],
                              xr_d.ap()[s0 - 1][:, 0:half])
        for s0 in (1, 2, 3, 4):
            nc.sync.dma_start(ROTS[:, s0 * BLOC + half:(s0 + 1) * BLOC],
                              xr_d.ap()[s0 - 1][:, half:BLOC])
        plan = ["c0", "c1", 5, "c2", 6, "c3", 7, "c4", 8, "c5",
                9, "c6", 10, "c7", 11, "c8", 12, "c9", 13, "c10", 14, "c11",
                15, "c12", 16, 17, 18, 19]
        for item in plan:
            if isinstance(item, str):
                dma_coef(int(item[1:]))
            else:
                dma_rot(item)

        nbt = BLOC // BT
        psums = []
        for bt in range(nbt):
            ps = psum_pool.tile([E, BT], f32, tag=f"ps{bt}", name=f"ps{bt}")
            psums.append(ps)

        # linear term first so the accumulation chain can start immediately
        for bt in range(nbt):
            nc.tensor.matmul(psums[bt][:, :], UM[:, :],
                             ROTS[:, bt * BT:bt * BT + BT],
                             start=True, stop=False)

        def emit_matmuls(pos, rhs_tile, col_base):
            ci, cc = divmod(pos, CHUNK)
            for bt in range(nbt):
                nc.tensor.matmul(
                    psums[bt][:, :],
                    coef_tiles[ci][:, cc * E:(cc + 1) * E],
                    rhs_tile[:, col_base + bt * BT:col_base + bt * BT + BT],
                    start=False,
                    stop=(pos == NPACK - 1),
                )

        # pack 0: x^2 on the scalar engine
        PK0 = p0_pool.tile([D, BLOC], b16)
        nc.scalar.square(PK0[:, :], ROTS[:, 0:BLOC])
        emit_matmuls(0, PK0, 0)

        # 16 groups of 4: in0 = rotations 0..3, in1 = rotation 4(k+1);
        # the first group runs in two batch-halves so it can start on the
        # half-width head DMAs
        for k in range(16):
            PKg = g_pool.tile([D, 4 * BLOC], b16, name=f"PKg{k}", tag="g4")
            P3 = PKg[:, :].rearrange("p (s b) -> p s b", s=4)
            splits = ((0, BLOC // 2), (BLOC // 2, BLOC)) if k == 0 \
                else ((0, BLOC),)
            for lo, hi in splits:
                nc.vector.tensor_mul(
                    P3[:, :, lo:hi],
                    R3[:, 0:4, lo:hi],
                    R3[:, 4 + k:5 + k, lo:hi].broadcast_to((D, 4, hi - lo)),
                )
            for i in range(4):
                emit_matmuls(1 + 4 * k + i, PKg, i * BLOC)

        for bt in range(nbt):
            OT = out_pool.tile([E, BT], f32)
            nc.scalar.activation(OT[:, :], psums[bt][:, :], Ident, bias=TV[:, 0:1])
            nc.sync.dma_start(out_d.ap()[:, bt * BT:(bt + 1) * BT], OT[:, :])

    nc.compile()
    return nc


def _host_precompute(Centroids: np.ndarray, Sigmas: np.ndarray):
    """Coefficient packs from the (replicated) small parameters."""
    Sinv = np.linalg.inv(Sigmas.astype(np.float64))
    A = 0.5 * (Sinv + np.swapaxes(Sinv, 1, 2))          # [E, D, D] symmetric
    c = Centroids[:, 0, :].astype(np.float64)           # [E, D]
    Ac = np.einsum("edk,ek->ed", A, c)

    cw = np.zeros((D, NPACK, E), np.float32)            # [row, emission pos, e]
    idx = np.arange(D)
    for pos, j in enumerate(ORDER):
        s = 2.0 if 1 <= j <= 63 else 1.0
        # row rotation of the product feeding this pack (the in0 slot index)
        a = 0 if j % 4 == 0 else 4 - (j % 4)
        cw[:, pos, :] = s * A[:, (idx + a) % D, (idx + a + j) % D].T
    cw_host = np.ascontiguousarray(cw.reshape(D, NPACK * E)).astype(bf16)
    um_host = np.ascontiguousarray((-2.0 * Ac.T)).astype(bf16)          # [D, E]
    tv_host = np.ascontiguousarray(
        np.einsum("ed,ed->e", Ac, c).astype(np.float32)[:, None]
    )                                                                    # [E, 1]
    return cw_host, um_host, tv_host


def _get_nc():
    if "nc" not in _STATE:
        os.environ.setdefault("JAX_COMPILATION_CACHE_DIR", "/root/.jax_cache")
        _STATE["nc"] = _build_module()
    return _STATE["nc"]


def _make_in_maps(x, Centroids, Sigmas):
    cw_host, um_host, tv_host = _host_precompute(
        np.asarray(Centroids, np.float32), np.asarray(Sigmas, np.float32)
    )
    xT = np.ascontiguousarray(np.asarray(x, np.float32).T).astype(bf16)  # [D, B]
    in_maps = []
    for cidx in range(NCORES):
        xTs = np.ascontiguousarray(xT[:, cidx * BLOC:(cidx + 1) * BLOC])
        xrot = np.stack([np.roll(xTs, -r, axis=0) for r in ROTVALS])
        in_maps.append({
            "xT": xTs,
            "xrot": np.ascontiguousarray(xrot),
            "cw": cw_host,
            "um": um_host,
            "tv": tv_host,
        })
    return in_maps


def _run_device(in_maps, trace=False):
    from concourse import bass_utils

    nc = _get_nc()
    return bass_utils.run_bass_kernel_spmd(
        nc, in_maps, core_ids=list(range(NCORES)), trace=trace
    )


def kernel(x, Centroids, Sigmas):
    in_maps = _make_in_maps(x, Centroids, Sigmas)
    res = _run_device(in_maps)
    outT = np.concatenate([res.results[c]["out"] for c in range(NCORES)], axis=1)
    return np.ascontiguousarray(outT.T).astype(np.float32)


# revision 15
# speedup vs baseline: 1.0112x; 1.0112x over previous
"""Trainium2 Bass kernel for the DEN (Mahalanobis distance) layer.

Computes out[b, e] = (x_b - c_e)^T Sigma_e^{-1} (x_b - c_e) for
x [8192, 128], Centroids [128, 1, 128], Sigmas [128, 128, 128].

Strategy
--------
Expand the quadratic form with A_e = sym(Sigma_e^{-1}) (the quadratic form
only sees the symmetric part):

    out[b, e] = x_b^T A_e x_b - 2 (A_e c_e) . x_b + c_e^T A_e c_e

and decompose x^T A x over the wrapped diagonals of A:

    x^T A_e x = sum_{j=0..64} sum_d s_j A_e[d, (d+j)%128] * x_d * x_{(d+j)%128}

(s_j = 2 for 1<=j<=63, else 1: each unordered pair lands in exactly one
wrapped diagonal j<=63, and diagonal 64 visits its pairs twice).

The shifted products P_j[d, b] = xT[d, b] * xT[(d+j)%128, b] feed a chain
of PSUM-accumulated [128,128]x[128,512] matmuls with host-precomputed
coefficient packs, one per diagonal, plus one matmul for the linear term;
the constant term rides in as the activation bias during PSUM->SBUF
eviction.

Compute-engine instructions need all operands on the same partitions
(lanes are hardwired per partition), so the shifts come from host-prebuilt
partition-rotated copies of xT: a difference set of rotations
{0..7, 8, 16, ..., 64} covers every j = b - a in 0..64, and the row
rotation by a_j is absorbed into the coefficient packs on the host.  All
rotations live in one SBUF mega-tile, so diagonals sharing an operand
merge into a single strided vector-engine op: the whole product stream is
16 four-diagonal tensor_mul ops + 1 scalar-engine square (j=0).

Sharding: data-parallel over batch B across the 8 cores (1024 rows each);
coefficient packs (derived from Sigmas/Centroids) are replicated.
"""

import os
import sys

sys.path.insert(0, "/opt/trn_rl_repo")

import numpy as np
import ml_dtypes

E, B, D = 128, 8192, 128
NCORES = 8
BLOC = B // NCORES          # 1024 batch rows per core
BT = 512                    # matmul free-dim tile (one PSUM bank)
NPACK = 65                  # wrapped diagonals j = 0..64
NSLOT = 20                  # rotation slots: 0..3 then 4,8,...,64
ROTVALS = (1, 2, 3) + tuple(range(4, 65, 4))
CHUNK = 5                   # coefficient packs per DMA chunk
NCHUNK = 13                 # 65 / 5

# pack emission order: j=0 (scalar-engine square), then 16 groups of 4:
# group k holds j = 4(k+1) - i for i = 0..3 (in0 = rotations 0..3,
# in1 = rotation 4(k+1) broadcast)
ORDER = [0] + [4 * (k + 1) - i for k in range(16) for i in range(4)]

bf16 = ml_dtypes.bfloat16

_STATE: dict = {}


def _build_module():
    import concourse.bacc as bacc
    import concourse.tile as tile
    import concourse.mybir as mybir
    from contextlib import ExitStack

    nc = bacc.Bacc("TRN2", target_bir_lowering=False, debug=False)

    xT_d = nc.dram_tensor("xT", [D, BLOC], mybir.dt.bfloat16, kind="ExternalInput")
    xr_d = nc.dram_tensor("xrot", [NSLOT - 1, D, BLOC], mybir.dt.bfloat16,
                          kind="ExternalInput")
    cw_d = nc.dram_tensor("cw", [D, NPACK * E], mybir.dt.bfloat16, kind="ExternalInput")
    um_d = nc.dram_tensor("um", [D, E], mybir.dt.bfloat16, kind="ExternalInput")
    tv_d = nc.dram_tensor("tv", [E, 1], mybir.dt.float32, kind="ExternalInput")
    out_d = nc.dram_tensor("out", [E, BLOC], mybir.dt.float32, kind="ExternalOutput")

    f32 = mybir.dt.float32
    b16 = mybir.dt.bfloat16
    Ident = mybir.ActivationFunctionType.Identity

    with tile.TileContext(nc) as tc, ExitStack() as ctx:
        const_pool = ctx.enter_context(tc.tile_pool(name="const", bufs=1))
        coef_pool = ctx.enter_context(tc.tile_pool(name="coef", bufs=NCHUNK))
        p0_pool = ctx.enter_context(tc.tile_pool(name="p0", bufs=1))
        g_pool = ctx.enter_context(tc.tile_pool(name="g", bufs=4))
        psum_pool = ctx.enter_context(tc.tile_pool(name="acc", bufs=1, space="PSUM"))
        out_pool = ctx.enter_context(tc.tile_pool(name="outs", bufs=2))

        ROTS = const_pool.tile([D, NSLOT * BLOC], b16, tag="rots")
        R3 = ROTS[:, :].rearrange("p (s b) -> p s b", s=NSLOT)
        UM = const_pool.tile([D, E], b16, tag="um")
        TV = const_pool.tile([E, 1], f32, tag="tv")
        coef_tiles = [coef_pool.tile([D, CHUNK * E], b16, name=f"cw{ci}", tag="cw")
                      for ci in range(NCHUNK)]

        # DMA emission in consumption order, rotations interleaved with
        # coefficient chunks so neither engine starves at the head
        nc.sync.dma_start(ROTS[:, 0:BLOC], xT_d.ap())
        nc.sync.dma_start(UM[:], um_d.ap())
        nc.sync.dma_start(TV[:], tv_d.ap())

        def dma_rot(s):
            nc.sync.dma_start(ROTS[:, s * BLOC:(s + 1) * BLOC], xr_d.ap()[s - 1])

        def dma_coef(ci):
            nc.sync.dma_start(coef_tiles[ci][:],
                              cw_d.ap()[:, ci * CHUNK * E:(ci + 1) * CHUNK * E])

        plan = ["c0", 1, 2, 3, 4, "c1", 5, "c2", 6, "c3", 7, "c4", 8, "c5",
                9, "c6", 10, "c7", 11, "c8", 12, "c9", 13, "c10", 14, "c11",
                15, "c12", 16, 17, 18, 19]
        for item in plan:
            if isinstance(item, str):
                dma_coef(int(item[1:]))
            else:
                dma_rot(item)

        nbt = BLOC // BT
        psums = []
        for bt in range(nbt):
            ps = psum_pool.tile([E, BT], f32, tag=f"ps{bt}", name=f"ps{bt}")
            psums.append(ps)

        # linear term first so the accumulation chain can start immediately
        for bt in range(nbt):
            nc.tensor.matmul(psums[bt][:, :], UM[:, :],
                             ROTS[:, bt * BT:bt * BT + BT],
                             start=True, stop=False)

        def emit_matmuls(pos, rhs_tile, col_base):
            ci, cc = divmod(pos, CHUNK)
            for bt in range(nbt):
                nc.tensor.matmul(
                    psums[bt][:, :],
                    coef_tiles[ci][:, cc * E:(cc + 1) * E],
                    rhs_tile[:, col_base + bt * BT:col_base + bt * BT + BT],
                    start=False,
                    stop=(pos == NPACK - 1),
                )

        # pack 0: x^2 on the scalar engine
        PK0 = p0_pool.tile([D, BLOC], b16)
        nc.scalar.square(PK0[:, :], ROTS[:, 0:BLOC])
        emit_matmuls(0, PK0, 0)

        # 16 groups of 4: in0 = rotations 0..3, in1 = rotation 4(k+1)
        for k in range(16):
            PKg = g_pool.tile([D, 4 * BLOC], b16, name=f"PKg{k}", tag="g4")
            nc.vector.tensor_mul(
                PKg[:, :].rearrange("p (s b) -> p s b", s=4),
                R3[:, 0:4, :],
                R3[:, 4 + k:5 + k, :].broadcast_to((D, 4, BLOC)),
            )
            for i in range(4):
                emit_matmuls(1 + 4 * k + i, PKg, i * BLOC)

        for bt in range(nbt):
            OT = out_pool.tile([E, BT], f32)
            nc.scalar.activation(OT[:, :], psums[bt][:, :], Ident, bias=TV[:, 0:1])
            nc.sync.dma_start(out_d.ap()[:, bt * BT:(bt + 1) * BT], OT[:, :])

    nc.compile()
    return nc


def _host_precompute(Centroids: np.ndarray, Sigmas: np.ndarray):
    """Coefficient packs from the (replicated) small parameters."""
    Sinv = np.linalg.inv(Sigmas.astype(np.float64))
    A = 0.5 * (Sinv + np.swapaxes(Sinv, 1, 2))          # [E, D, D] symmetric
    c = Centroids[:, 0, :].astype(np.float64)           # [E, D]
    Ac = np.einsum("edk,ek->ed", A, c)

    cw = np.zeros((D, NPACK, E), np.float32)            # [row, emission pos, e]
    idx = np.arange(D)
    for pos, j in enumerate(ORDER):
        s = 2.0 if 1 <= j <= 63 else 1.0
        # row rotation of the product feeding this pack (the in0 slot index)
        a = 0 if j % 4 == 0 else 4 - (j % 4)
        cw[:, pos, :] = s * A[:, (idx + a) % D, (idx + a + j) % D].T
    cw_host = np.ascontiguousarray(cw.reshape(D, NPACK * E)).astype(bf16)
    um_host = np.ascontiguousarray((-2.0 * Ac.T)).astype(bf16)          # [D, E]
    tv_host = np.ascontiguousarray(
        np.einsum("ed,ed->e", Ac, c).astype(np.float32)[:, None]
    )                                                                    # [E, 1]
    return cw_host, um_host, tv_host


def _get_nc():
    if "nc" not in _STATE:
        os.environ.setdefault("JAX_COMPILATION_CACHE_DIR", "/root/.jax_cache")
        _STATE["nc"] = _build_module()
    return _STATE["nc"]


def _make_in_maps(x, Centroids, Sigmas):
    cw_host, um_host, tv_host = _host_precompute(
        np.asarray(Centroids, np.float32), np.asarray(Sigmas, np.float32)
    )
    xT = np.ascontiguousarray(np.asarray(x, np.float32).T).astype(bf16)  # [D, B]
    in_maps = []
    for cidx in range(NCORES):
        xTs = np.ascontiguousarray(xT[:, cidx * BLOC:(cidx + 1) * BLOC])
        xrot = np.stack([np.roll(xTs, -r, axis=0) for r in ROTVALS])
        in_maps.append({
            "xT": xTs,
            "xrot": np.ascontiguousarray(xrot),
            "cw": cw_host,
            "um": um_host,
            "tv": tv_host,
        })
    return in_maps


def _run_device(in_maps, trace=False):
    from concourse import bass_utils

    nc = _get_nc()
    return bass_utils.run_bass_kernel_spmd(
        nc, in_maps, core_ids=list(range(NCORES)), trace=trace
    )


def kernel(x, Centroids, Sigmas):
    in_maps = _make_in_maps(x, Centroids, Sigmas)
    res = _run_device(in_maps)
    outT = np.concatenate([res.results[c]["out"] for c in range(NCORES)], axis=1)
    return np.ascontiguousarray(outT.T).astype(np.float32)


# revision 17
# speedup vs baseline: 1.0377x; 1.0263x over previous
"""Trainium2 Bass kernel for the DEN (Mahalanobis distance) layer.

Computes out[b, e] = (x_b - c_e)^T Sigma_e^{-1} (x_b - c_e) for
x [8192, 128], Centroids [128, 1, 128], Sigmas [128, 128, 128].

Strategy
--------
Expand the quadratic form with A_e = sym(Sigma_e^{-1}) (the quadratic form
only sees the symmetric part):

    out[b, e] = x_b^T A_e x_b - 2 (A_e c_e) . x_b + c_e^T A_e c_e

and decompose x^T A x over the wrapped diagonals of A:

    x^T A_e x = sum_{j=0..64} sum_d s_j A_e[d, (d+j)%128] * x_d * x_{(d+j)%128}

(s_j = 2 for 1<=j<=63, else 1: each unordered pair lands in exactly one
wrapped diagonal j<=63, and diagonal 64 visits its pairs twice).

The shifted products P_j[d, b] = xT[d, b] * xT[(d+j)%128, b] feed a chain
of PSUM-accumulated [128,128]x[128,512] matmuls with host-precomputed
coefficient packs, one per diagonal, plus one matmul for the linear term;
the constant term rides in as the activation bias during PSUM->SBUF
eviction.

Compute-engine instructions need all operands on the same partitions
(lanes are hardwired per partition), so the shifts come from host-prebuilt
partition-rotated copies of xT: a difference set of rotations
{0..7, 8, 16, ..., 64} covers every j = b - a in 0..64, and the row
rotation by a_j is absorbed into the coefficient packs on the host.  All
rotations live in one SBUF mega-tile, so diagonals sharing an operand
merge into a single strided vector-engine op: the whole product stream is
16 four-diagonal tensor_mul ops + 1 scalar-engine square (j=0).

Sharding: data-parallel over batch B across the 8 cores (1024 rows each);
coefficient packs (derived from Sigmas/Centroids) are replicated.
"""

import os
import sys

sys.path.insert(0, "/opt/trn_rl_repo")

import numpy as np
import ml_dtypes

E, B, D = 128, 8192, 128
NCORES = 8
BLOC = B // NCORES          # 1024 batch rows per core
BT = 512                    # matmul free-dim tile (one PSUM bank)
NPACK = 65                  # wrapped diagonals j = 0..64
NSLOT = 16                  # rotation slots: 0..7 then 8,16,...,64
ROTVALS = (1, 2, 3, 4, 5, 6, 7) + tuple(range(8, 65, 8))
CHUNK = 5                   # coefficient packs per DMA chunk
NCHUNK = 13                 # 65 / 5

# mixed-radix product groups: (in0 slot range [lo,hi), in1 slot).
# Column i of a group is the product rot[lo+i] * rot[in1], covering
# diagonal j = rotval(in1) - rotval(lo+i) with row rotation a = lo+i.
# Tiny groups at the head (fewest DMA prerequisites), radix-8 in the
# middle, tiny group last (fast tensor-engine drain).
GROUPS = [(0, 1, 1), (0, 1, 2), (0, 2, 4), (0, 4, 8)] \
         + [(0, 8, 8 + k) for k in range(1, 7)] \
         + [(2, 8, 15), (0, 2, 15)]
def _slotval(s):
    return s if s <= 7 else 8 * (s - 7)


# pack emission order: j=0 (scalar-engine square), then the group columns;
# AVAL[pos] is the row rotation of the product feeding that pack
ORDER = [0]
AVAL = [0]
for _lo, _hi, _s1 in GROUPS:
    for _i in range(_hi - _lo):
        ORDER.append(_slotval(_s1) - (_lo + _i))
        AVAL.append(_lo + _i)

bf16 = ml_dtypes.bfloat16

_STATE: dict = {}


def _build_module():
    import concourse.bacc as bacc
    import concourse.tile as tile
    import concourse.mybir as mybir
    from contextlib import ExitStack

    nc = bacc.Bacc("TRN2", target_bir_lowering=False, debug=False)

    xT_d = nc.dram_tensor("xT", [D, BLOC], mybir.dt.bfloat16, kind="ExternalInput")
    xr_d = nc.dram_tensor("xrot", [NSLOT - 1, D, BLOC], mybir.dt.bfloat16,
                          kind="ExternalInput")
    cw_d = nc.dram_tensor("cw", [D, NPACK * E], mybir.dt.bfloat16, kind="ExternalInput")
    um_d = nc.dram_tensor("um", [D, E], mybir.dt.bfloat16, kind="ExternalInput")
    tv_d = nc.dram_tensor("tv", [E, 1], mybir.dt.float32, kind="ExternalInput")
    out_d = nc.dram_tensor("out", [E, BLOC], mybir.dt.float32, kind="ExternalOutput")

    f32 = mybir.dt.float32
    b16 = mybir.dt.bfloat16
    Ident = mybir.ActivationFunctionType.Identity

    with tile.TileContext(nc) as tc, ExitStack() as ctx:
        const_pool = ctx.enter_context(tc.tile_pool(name="const", bufs=1))
        coef_pool = ctx.enter_context(tc.tile_pool(name="coef", bufs=NCHUNK))
        p0_pool = ctx.enter_context(tc.tile_pool(name="p0", bufs=1))
        g_pool = ctx.enter_context(tc.tile_pool(name="g", bufs=5))
        psum_pool = ctx.enter_context(tc.tile_pool(name="acc", bufs=1, space="PSUM"))
        out_pool = ctx.enter_context(tc.tile_pool(name="outs", bufs=2))

        ROTS = const_pool.tile([D, NSLOT * BLOC], b16, tag="rots")
        R3 = ROTS[:, :].rearrange("p (s b) -> p s b", s=NSLOT)
        UM = const_pool.tile([D, E], b16, tag="um")
        TV = const_pool.tile([E, 1], f32, tag="tv")
        coef_tiles = [coef_pool.tile([D, CHUNK * E], b16, name=f"cw{ci}", tag="cw")
                      for ci in range(NCHUNK)]

        # DMA emission in consumption order, rotations interleaved with
        # coefficient chunks so neither engine starves at the head
        nc.sync.dma_start(ROTS[:, 0:BLOC], xT_d.ap())
        nc.sync.dma_start(UM[:], um_d.ap())
        nc.sync.dma_start(TV[:], tv_d.ap())

        def dma_rot(s):
            nc.sync.dma_start(ROTS[:, s * BLOC:(s + 1) * BLOC], xr_d.ap()[s - 1])

        def dma_coef(ci):
            nc.sync.dma_start(coef_tiles[ci][:],
                              cw_d.ap()[:, ci * CHUNK * E:(ci + 1) * CHUNK * E])

        plan = [1, 2, 4, 3, 8, 5, 6, 7, 9, "c0", 10, "c1", 11, "c2", 12,
                "c3", 13, "c4", 14, "c5", 15, "c6", "c7", "c8", "c9", "c10",
                "c11", "c12"]
        for item in plan:
            if isinstance(item, str):
                dma_coef(int(item[1:]))
            else:
                dma_rot(item)

        nbt = BLOC // BT
        psums = []
        for bt in range(nbt):
            ps = psum_pool.tile([E, BT], f32, tag=f"ps{bt}", name=f"ps{bt}")
            psums.append(ps)

        # linear term first so the accumulation chain can start immediately
        for bt in range(nbt):
            nc.tensor.matmul(psums[bt][:, :], UM[:, :],
                             ROTS[:, bt * BT:bt * BT + BT],
                             start=True, stop=False)

        def emit_matmuls(pos, rhs_tile, col_base):
            ci, cc = divmod(pos, CHUNK)
            for bt in range(nbt):
                nc.tensor.matmul(
                    psums[bt][:, :],
                    coef_tiles[ci][:, cc * E:(cc + 1) * E],
                    rhs_tile[:, col_base + bt * BT:col_base + bt * BT + BT],
                    start=False,
                    stop=(pos == NPACK - 1),
                )

        # pack 0: x^2 on the scalar engine
        PK0 = p0_pool.tile([D, BLOC], b16)
        nc.scalar.square(PK0[:, :], ROTS[:, 0:BLOC])
        emit_matmuls(0, PK0, 0)

        # mixed-radix product groups
        pos = 1
        for gi, (lo, hi, s1) in enumerate(GROUPS):
            w = hi - lo
            PKg = g_pool.tile([D, w * BLOC], b16, name=f"PKg{gi}", tag="g")
            nc.vector.tensor_mul(
                PKg[:, :].rearrange("p (s b) -> p s b", s=w),
                R3[:, lo:hi, :],
                R3[:, s1:s1 + 1, :].broadcast_to((D, w, BLOC)),
            )
            for i in range(w):
                emit_matmuls(pos, PKg, i * BLOC)
                pos += 1

        for bt in range(nbt):
            OT = out_pool.tile([E, BT], f32)
            if bt == 0:
                nc.vector.tensor_scalar_add(OT[:, :], psums[bt][:, :], TV[:, 0:1])
            else:
                nc.scalar.activation(OT[:, :], psums[bt][:, :], Ident,
                                     bias=TV[:, 0:1])
            nc.sync.dma_start(out_d.ap()[:, bt * BT:(bt + 1) * BT], OT[:, :])

    nc.compile()
    return nc


def _host_precompute(Centroids: np.ndarray, Sigmas: np.ndarray):
    """Coefficient packs from the (replicated) small parameters."""
    Sinv = np.linalg.inv(Sigmas.astype(np.float64))
    A = 0.5 * (Sinv + np.swapaxes(Sinv, 1, 2))          # [E, D, D] symmetric
    c = Centroids[:, 0, :].astype(np.float64)           # [E, D]
    Ac = np.einsum("edk,ek->ed", A, c)

    cw = np.zeros((D, NPACK, E), np.float32)            # [row, emission pos, e]
    idx = np.arange(D)
    for pos, j in enumerate(ORDER):
        s = 2.0 if 1 <= j <= 63 else 1.0
        a = AVAL[pos]
        cw[:, pos, :] = s * A[:, (idx + a) % D, (idx + a + j) % D].T
    cw_host = np.ascontiguousarray(cw.reshape(D, NPACK * E)).astype(bf16)
    um_host = np.ascontiguousarray((-2.0 * Ac.T)).astype(bf16)          # [D, E]
    tv_host = np.ascontiguousarray(
        np.einsum("ed,ed->e", Ac, c).astype(np.float32)[:, None]
    )                                                                    # [E, 1]
    return cw_host, um_host, tv_host


def _get_nc():
    if "nc" not in _STATE:
        os.environ.setdefault("JAX_COMPILATION_CACHE_DIR", "/root/.jax_cache")
        _STATE["nc"] = _build_module()
    return _STATE["nc"]


def _make_in_maps(x, Centroids, Sigmas):
    cw_host, um_host, tv_host = _host_precompute(
        np.asarray(Centroids, np.float32), np.asarray(Sigmas, np.float32)
    )
    xT = np.ascontiguousarray(np.asarray(x, np.float32).T).astype(bf16)  # [D, B]
    in_maps = []
    for cidx in range(NCORES):
        xTs = np.ascontiguousarray(xT[:, cidx * BLOC:(cidx + 1) * BLOC])
        xrot = np.stack([np.roll(xTs, -r, axis=0) for r in ROTVALS])
        in_maps.append({
            "xT": xTs,
            "xrot": np.ascontiguousarray(xrot),
            "cw": cw_host,
            "um": um_host,
            "tv": tv_host,
        })
    return in_maps


def _run_device(in_maps, trace=False):
    from concourse import bass_utils

    nc = _get_nc()
    return bass_utils.run_bass_kernel_spmd(
        nc, in_maps, core_ids=list(range(NCORES)), trace=trace
    )


def kernel(x, Centroids, Sigmas):
    in_maps = _make_in_maps(x, Centroids, Sigmas)
    res = _run_device(in_maps)
    outT = np.concatenate([res.results[c]["out"] for c in range(NCORES)], axis=1)
    return np.ascontiguousarray(outT.T).astype(np.float32)
